# revision 6
# baseline (speedup 1.0000x reference)
"""DSQG sparse attention kernel for 8 Trainium2 NeuronCores.

Problem: B=2, T=2048, C=768, H=12, HD=64, J=52 offsets (41 dense 0..40 + 11 sparse).
out = softmax_j(q . (k[t-oj] * (1+se[j])) / 8 + pb[j,h]) @ v[t-oj], then out-proj.

Sharding (SPMD, one program, 8 input sets):
  core c: b = c//4, th = (c%4)//2 (T-half), hg = (c%4)%2 (head-group of 6).
  Queries t in [th*1024, th*1024+1024), K/V halo [t0-384, t0+1024) zero-padded.
  Host sums the hg partials per (b, th) and concatenates.

Per-core pipeline (emission interleaved across packs for engine overlap):
  P1 PE  : qk-proj -> QT/KT packs [128=(2h x 64d), t] bf16; v-proj -> V [t%128, blk, d'].
  P2 DVE : per offset j: prod_j = QT .* KT[:, shifted]  (bf16 TT)
     PE  : scores += dse_j.T @ prod_j  (lhsT [128,32], accumulate in PSUM [128, 512])
     ACT : EP = exp(scores/8 + pb)  -> [128=(4cg x 32slot), t] bf16; DVE: validity mask.
     PE  : colsum EP via head-select lhsT [128,2] -> denominators [2, 512] x2.
     DVE : reciprocal [2,1024]; Pool: partition_broadcast -> rbc [64, 1024] per head.
  P3 PE  : transpose EP 4-tau groups -> pt4 [t, 128rows] (1 ACT copy per group).
  P4 GPS : local_scatter both heads -> band2 [t, 1024] (diagonal placement).
  P5 PE  : transpose band chunks -> bt [s, t].
  P6 PE  : AV: psV[64h.., tau] += V-chunk.T @ bt; DVE stt: OHT = psV * rbc.
  P7 PE  : out-proj: OUT[t, 768] = OHT.T @ WoT (if_gain folded into WoT). DMA out f32.
"""
import sys
sys.path.insert(0, "/opt/trn_rl_repo")

import numpy as np
import ml_dtypes

BF16 = ml_dtypes.bfloat16

B, T, C, H, HD = 2, 2048, 768, 12, 64
J = 52
OFFS = np.array(list(range(41)) + [96, 128, 145, 163, 185, 209, 236, 266, 301, 340, 384],
                dtype=np.int32)
NUM_LOCAL_HEADS = 7
DISTAL_THRESHOLD = 350.0
TQ = 1024          # queries per core
HALO = 384
TK = TQ + HALO     # 1408
HPC = 6            # heads per core
NPACK = 3          # head pairs per core

_compiled = None


def _build():
    import concourse.bass as bass
    import concourse.tile as tile
    from concourse import mybir, bacc
    from concourse.masks import make_identity

    nc = bacc.Bacc()
    f32, bf16, i16 = mybir.dt.float32, mybir.dt.bfloat16, mybir.dt.int16

    xt = nc.dram_tensor("xt", [768, TK], bf16, kind="ExternalInput")
    wqk = nc.dram_tensor("wqk", [768, 768], bf16, kind="ExternalInput")
    wv = nc.dram_tensor("wv", [768, 384], bf16, kind="ExternalInput")
    wo = nc.dram_tensor("wo", [384, 768], bf16, kind="ExternalInput")
    dse = nc.dram_tensor("dse", [J, 128, 32], bf16, kind="ExternalInput")
    pb = nc.dram_tensor("pb", [128, NPACK], f32, kind="ExternalInput")
    vmask = nc.dram_tensor("vmask", [128, 512], bf16, kind="ExternalInput")
    sidx2 = nc.dram_tensor("sidx2", [128, 128], i16, kind="ExternalInput")
    hsel = nc.dram_tensor("hsel", [128, 33], bf16, kind="ExternalInput")
    out_d = nc.dram_tensor("out", [TQ, 768], f32, kind="ExternalOutput")

    NT = TQ // 128   # 8 query tiles
    NB = TK // 128   # 11 halo blocks

    with tile.TileContext(nc) as tc:
        import contextlib
        with contextlib.ExitStack() as ctx:
            consts = ctx.enter_context(tc.tile_pool(name="consts", bufs=1))
            qkv = ctx.enter_context(tc.tile_pool(name="qkv", bufs=1))
            prodp = ctx.enter_context(tc.tile_pool(name="prod", bufs=8))
            epp = ctx.enter_context(tc.tile_pool(name="ep", bufs=1))
            ptp = ctx.enter_context(tc.tile_pool(name="pt", bufs=3))
            bandp = ctx.enter_context(tc.tile_pool(name="band", bufs=6))
            btp = ctx.enter_context(tc.tile_pool(name="bandT", bufs=6))
            ohp = ctx.enter_context(tc.tile_pool(name="oh", bufs=1))
            outp = ctx.enter_context(tc.tile_pool(name="outsb", bufs=3))
            smallp = ctx.enter_context(tc.tile_pool(name="small", bufs=8))
            rbcp = ctx.enter_context(tc.tile_pool(name="rbc", bufs=6))
            psA = ctx.enter_context(tc.tile_pool(name="psA", bufs=2, space="PSUM"))
            psS = ctx.enter_context(tc.tile_pool(name="psS", bufs=2, space="PSUM"))
            psT = ctx.enter_context(tc.tile_pool(name="psT", bufs=2, space="PSUM"))
            psV = ctx.enter_context(tc.tile_pool(name="psV", bufs=2, space="PSUM"))

            # ---- chunked constant loads: first-needed first ----
            wqk_sb = consts.tile([128, 6, 768], bf16)
            xt_sb = consts.tile([128, 6, TK], bf16)
            wqk_r = wqk.rearrange("(a p) m -> p a m", p=128)
            xt_r = xt.rearrange("(a p) t -> p a t", p=128)
            for mt in (0, 3):
                nc.sync.dma_start(out=wqk_sb[:, :, mt * 128:(mt + 1) * 128],
                                  in_=wqk_r[:, :, mt * 128:(mt + 1) * 128])
            for a in range(6):
                nc.sync.dma_start(out=xt_sb[:, a, :], in_=xt_r[:, a, :])
            for mt in (1, 4, 2, 5):
                nc.sync.dma_start(out=wqk_sb[:, :, mt * 128:(mt + 1) * 128],
                                  in_=wqk_r[:, :, mt * 128:(mt + 1) * 128])
            dse_sb = consts.tile([128, J, 32], bf16)
            nc.sync.dma_start(out=dse_sb, in_=dse.rearrange("j p m -> p j m"))
            pb_sb = consts.tile([128, NPACK], f32)
            nc.sync.dma_start(out=pb_sb, in_=pb[:])
            vmask_sb = consts.tile([128, 512], bf16)
            nc.sync.dma_start(out=vmask_sb, in_=vmask[:])
            sidx_sb = consts.tile([128, 128], i16)
            nc.sync.dma_start(out=sidx_sb, in_=sidx2[:])
            hsel_sb = consts.tile([128, 33], bf16)
            nc.sync.dma_start(out=hsel_sb, in_=hsel[:])
            wv_sb = consts.tile([128, 6, 384], bf16)
            nc.sync.dma_start(out=wv_sb, in_=wv.rearrange("(a p) m -> p a m", p=128))
            wo_sb = consts.tile([128, 3, 768], bf16)
            nc.sync.dma_start(out=wo_sb, in_=wo.rearrange("(a p) m -> p a m", p=128))
            ident = consts.tile([128, 128], bf16)
            make_identity(nc, ident)

            QT = qkv.tile([128, NPACK, TQ], bf16, tag="QT")
            KT = qkv.tile([128, NPACK, TK], bf16, tag="KT")
            V = qkv.tile([128, NB, 384], bf16, tag="V")
            EP = epp.tile([128, NPACK, TQ], bf16)
            oht0 = ohp.tile([128, TQ], bf16)
            oht1 = ohp.tile([128, TQ], bf16)
            oht2 = ohp.tile([128, TQ], bf16)
            OHT = [oht0, oht1, oht2]

            def emit_qkproj(mt):
                # m-tiles 0..2 = Q (t in [384,1408) only), 3..5 = K (full)
                if mt < 3:
                    nranges = [(384, 896), (896, 1408)]
                else:
                    nranges = [(0, 512), (512, 1024), (1024, 1408)]
                for (n0, n1) in nranges:
                    nw = n1 - n0
                    ps = psA.tile([128, 512], f32, tag="psA")
                    for kc in range(6):
                        nc.tensor.matmul(
                            ps[:, 0:nw],
                            wqk_sb[:, kc, mt * 128:(mt + 1) * 128],
                            xt_sb[:, kc, n0:n1],
                            start=(kc == 0), stop=(kc == 5))
                    if mt < 3:
                        nc.scalar.copy(QT[:, mt, n0 - 384:n1 - 384], ps[:, 0:nw])
                    else:
                        nc.scalar.copy(KT[:, mt - 3, n0:n1], ps[:, 0:nw])

            def emit_vproj():
                for tt in range(NB):
                    ps = psA.tile([128, 512], f32, tag="psA")
                    for kc in range(6):
                        nc.tensor.matmul(
                            ps[:, 0:384],
                            xt_sb[:, kc, tt * 128:(tt + 1) * 128],
                            wv_sb[:, kc, :],
                            start=(kc == 0), stop=(kc == 5))
                    nc.scalar.copy(V[:, tt, :], ps[:, 0:384])

            def emit_scores(p):
                """DVE prods + PE dse accumulation + exp + vmask for pack p."""
                sps0 = psS.tile([128, 512], f32, tag="psS")
                sps1 = psS.tile([128, 512], f32, tag="psS")
                sps = [sps0, sps1]
                for q in range(13):
                    for cg in range(4):
                        jj = 13 * cg + q
                        oj = int(OFFS[jj])
                        prod = prodp.tile([128, TQ], bf16, tag="prod")
                        nc.vector.tensor_mul(
                            prod, QT[:, p, :], KT[:, p, HALO - oj:HALO - oj + TQ])
                        for n in range(2):
                            nc.tensor.matmul(
                                sps[n][32 * cg:32 * cg + 32, :],
                                dse_sb[:, jj, :],
                                prod[:, n * 512:(n + 1) * 512],
                                start=(q == 0), stop=(q == 12),
                                tile_position=(0, 32 * cg),
                                skip_group_check=True)
                for n in range(2):
                    nc.scalar.activation(
                        EP[:, p, n * 512:(n + 1) * 512], sps[n][:],
                        mybir.ActivationFunctionType.Exp,
                        bias=pb_sb[:, p:p + 1], scale=0.125)
                # validity mask only affects t < 512 (max offset 384)
                nc.vector.tensor_mul(EP[:, p, 0:512], EP[:, p, 0:512], vmask_sb[:])

            def emit_denoms(p):
                """Column-sum EP per head -> reciprocal -> broadcast to [64, t]."""
                rec = [smallp.tile([1, TQ], f32, tag="rec", name=f"rec{p}h{h}")
                       for h in range(2)]
                for n in range(2):
                    cs = psA.tile([128, 512], f32, tag="psA")
                    nc.tensor.matmul(cs[0:33, :], hsel_sb[:],
                                     EP[:, p, n * 512:(n + 1) * 512],
                                     start=True, stop=True)
                    for h in range(2):
                        nc.vector.reciprocal(
                            rec[h][:, n * 512:(n + 1) * 512],
                            cs[32 * h:32 * h + 1, :])
                rbc = [rbcp.tile([64, TQ], f32, tag="rbc", name=f"rbc{p}h{h}")
                       for h in range(2)]
                for h in range(2):
                    for n in range(2):
                        nc.gpsimd.partition_broadcast(
                            rbc[h][:, n * 512:(n + 1) * 512],
                            rec[h][:, n * 512:(n + 1) * 512], channels=64)
                return rbc

            def emit_av_group(p, g, rbc):
                """tau group g (4 tiles): EP-T, scatter, band-T, AV, normalize."""
                tpsE = psT.tile([128, 512], bf16, tag="psT")
                for ti in range(4):
                    tau = 4 * g + ti
                    nc.tensor.transpose(
                        tpsE[:, ti * 128:(ti + 1) * 128],
                        EP[:, p, tau * 128:(tau + 1) * 128], ident)
                pt4 = ptp.tile([128, 512], bf16, tag="pt")
                nc.scalar.copy(pt4, tpsE)
                psVt = psV.tile([128, 512], f32, tag="psV")
                for ti in range(4):
                    tau = 4 * g + ti
                    band2 = bandp.tile([128, 1024], bf16, tag="band")
                    nc.gpsimd.local_scatter(
                        out_ap=band2[:], data_ap=pt4[:, ti * 128:(ti + 1) * 128],
                        idxs_ap=sidx_sb[:], channels=128, num_elems=1024,
                        num_idxs=128)
                    for h in range(2):
                        hloc = 2 * p + h
                        btps = psT.tile([128, 512], bf16, tag="psT")
                        for c in range(4):
                            nc.tensor.transpose(
                                btps[:, c * 128:(c + 1) * 128],
                                band2[:, h * 512 + c * 128:h * 512 + (c + 1) * 128],
                                ident)
                        bt = btp.tile([128, 512], bf16, tag="bt")
                        nc.scalar.copy(bt, btps)
                        for c in range(4):
                            nc.tensor.matmul(
                                psVt[64 * h:64 * h + 64, ti * 128:(ti + 1) * 128],
                                V[:, tau + c, 64 * hloc:64 * hloc + 64],
                                bt[:, c * 128:(c + 1) * 128],
                                start=(c == 0), stop=(c == 3))
                for h in range(2):
                    nc.vector.scalar_tensor_tensor(
                        out=OHT[p][64 * h:64 * h + 64, g * 512:(g + 1) * 512],
                        in0=psVt[64 * h:64 * h + 64, :], scalar=1.0,
                        in1=rbc[h][:, g * 512:(g + 1) * 512],
                        op0=mybir.AluOpType.mult, op1=mybir.AluOpType.mult)

            def emit_outproj_group(g):
                for ti in range(4):
                    tau = 4 * g + ti
                    osb = outp.tile([128, 768], f32, tag="osb")
                    for (n0, n1) in [(0, 512), (512, 768)]:
                        nw = n1 - n0
                        ps = psA.tile([128, 512], f32, tag="psA")
                        for gg in range(3):
                            nc.tensor.matmul(
                                ps[:, 0:nw],
                                OHT[gg][:, tau * 128:(tau + 1) * 128],
                                wo_sb[:, gg, n0:n1],
                                start=(gg == 0), stop=(gg == 2))
                        nc.scalar.copy(osb[:, n0:n1], ps[:, 0:nw])
                    nc.sync.dma_start(
                        out=out_d[tau * 128:(tau + 1) * 128, :], in_=osb[:])

            # ---- interleaved emission ----
            emit_qkproj(0)
            emit_qkproj(3)
            emit_qkproj(1)
            emit_qkproj(4)
            emit_scores(0)
            emit_vproj()
            emit_qkproj(2)
            emit_qkproj(5)
            rbc0 = emit_denoms(0)
            emit_scores(1)
            rbc1 = emit_denoms(1)
            emit_av_group(0, 0, rbc0)
            emit_av_group(0, 1, rbc0)
            emit_scores(2)
            rbc2 = emit_denoms(2)
            emit_av_group(1, 0, rbc1)
            emit_av_group(1, 1, rbc1)
            emit_av_group(2, 0, rbc2)
            emit_outproj_group(0)
            emit_av_group(2, 1, rbc2)
            emit_outproj_group(1)

    nc.compile()
    return nc


def _host_prep(x, W_qkv, W_out, pos_bias, scale_embed, if_gain):
    """Build the 8 per-core input dicts."""
    delta = OFFS.astype(np.float32)
    distal = delta > DISTAL_THRESHOLD
    hidx = np.arange(H)
    pbm = np.where(distal[:, None] & (hidx[None, :] < NUM_LOCAL_HEADS), -10000.0,
                   pos_bias.astype(np.float32))
    pbm = np.where((~distal)[:, None] & (hidx[None, :] >= NUM_LOCAL_HEADS), -3.0, pbm)

    def mrow(jj, h):
        return 32 * (jj // 13) + 13 * h + (jj % 13)

    # sidx2[i, m] = i + 384 - o_j + 512*h if m == mrow(j, h) else -1 (ignored)
    sidx_np = np.full((128, 128), -1, dtype=np.int16)
    for h in range(2):
        for jj in range(J):
            sidx_np[:, mrow(jj, h)] = (np.arange(128) + HALO - OFFS[jj]
                                       + 512 * h).astype(np.int16)

    # hsel[p, 32*h] = 1 where row p belongs to head h (head 1 lands at
    # partition 32 so the reciprocal reads on a 32-partition boundary)
    hsel_np = np.zeros((128, 33), dtype=np.float32)
    for h in range(2):
        for jj in range(J):
            hsel_np[mrow(jj, h), 32 * h] = 1.0

    in_maps = []
    for c in range(8):
        b, q = divmod(c, 4)
        th, hg = divmod(q, 2)
        heads = np.arange(hg * HPC, hg * HPC + HPC)
        t0 = th * TQ

        # xt: [768, TK] halo-padded transpose of x[b]
        xt_np = np.zeros((768, TK), dtype=np.float32)
        lo = t0 - HALO
        src_lo = max(lo, 0)
        xt_np[:, src_lo - lo:] = x[b, src_lo:t0 + TQ, :].T
        # wqk: [768, 768] lhsT; cols 0..383 q-heads, 384..767 k-heads
        qrows = np.concatenate([np.arange(h * HD, (h + 1) * HD) for h in heads])
        wqk_np = np.concatenate(
            [W_qkv[qrows, :].T, W_qkv[768 + qrows, :].T], axis=1)
        wv_np = W_qkv[1536 + qrows, :].T
        # wo: [384, 768] lhsT for out-proj, if_gain folded
        gain = np.repeat(if_gain[heads], HD)
        wo_np = (W_out[:, qrows] * gain[None, :]).T
        # dse: [J, 128, 32] lhsT; cols 26..31 zero so PSUM pad rows read 0
        dse_np = np.zeros((J, 128, 32), dtype=np.float32)
        se1 = 1.0 + scale_embed.astype(np.float32)  # [J, HD]
        for jj in range(J):
            for h in range(2):
                dse_np[jj, h * 64:(h + 1) * 64, 13 * h + (jj % 13)] = se1[jj]
        # pb: [128, NPACK] bias columns per pack; pad rows -> -1e4 so exp -> 0
        pb_np = np.full((128, NPACK), -10000.0, dtype=np.float32)
        for p in range(NPACK):
            for h in range(2):
                for jj in range(J):
                    pb_np[mrow(jj, h), p] = pbm[jj, heads[2 * p + h]]
        # vmask [128, 512]: zero where global t < offset (th=0 only)
        vm = np.ones((128, 512), dtype=np.float32)
        if th == 0:
            tg = np.arange(512)
            for h in range(2):
                for jj in range(J):
                    vm[mrow(jj, h), :] = (tg >= OFFS[jj])
        in_maps.append({
            "xt": xt_np.astype(BF16),
            "wqk": wqk_np.astype(BF16),
            "wv": wv_np.astype(BF16),
            "wo": wo_np.astype(BF16),
            "dse": dse_np.astype(BF16),
            "pb": pb_np,
            "vmask": vm.astype(BF16),
            "sidx2": sidx_np,
            "hsel": hsel_np.astype(BF16),
        })
    return in_maps


def kernel(x, W_qkv, W_out, pos_bias, scale_embed, if_gain):
    global _compiled
    from concourse.bass_utils import run_bass_kernel_spmd

    x = np.asarray(x, dtype=np.float32)
    W_qkv = np.asarray(W_qkv, dtype=np.float32)
    W_out = np.asarray(W_out, dtype=np.float32)
    pos_bias = np.asarray(pos_bias, dtype=np.float32)
    scale_embed = np.asarray(scale_embed, dtype=np.float32)
    if_gain = np.asarray(if_gain, dtype=np.float32)

    if _compiled is None:
        _compiled = _build()
    in_maps = _host_prep(x, W_qkv, W_out, pos_bias, scale_embed, if_gain)
    res = run_bass_kernel_spmd(_compiled, in_maps, core_ids=list(range(8)))

    out = np.zeros((B, T, C), dtype=np.float32)
    for c in range(8):
        b, q = divmod(c, 4)
        th, _ = divmod(q, 2)
        t0 = th * TQ
        out[c // 4, t0:t0 + TQ, :] += res.results[c]["out"]
    return out
